# revision 26
# baseline (speedup 1.0000x reference)
"""BMMRemapper Trainium2 kernel.

Math: out[n,c,q] = sum_k x[n,c,k] * mat[n,q,k] where mat is the bilinear
interpolation matrix built from grid (4 nonzeros per row q: rows lin, lin+1,
lin+48, lin+49 of x^T with weights (1-a)(1-b), (1-a)b, a(1-b), ab).

Instead of a dense 2304x2304 BMM we exploit the 4-sparsity: the host stages
a quad-row table xq[k] = [x^T[k], x^T[k+1], x^T[k+48], x^T[k+49]] in fp16
(pure data movement + dtype cast), so ONE indirect-DMA descriptor per output
pixel fetches all four corner rows (1 KB contiguous). The HW vector-indirect
DMA consumes exactly one offset per partition per instruction (measured; the
simulator's multi-offset ravel semantics do NOT match HW), so the gather is
18 instructions of 128 descriptors each; their ~1.1-1.4 us SWDGE ucode
launches serialize on GPSIMD and set this kernel's floor (~25 us). The fp16
table halves the gathered bytes so the SDMA transfers, the combine (ACT: two
products via per-partition activation scale; DVE: two fused multiply-adds +
one add per tile) and the chunked output stores all hide under that wall.

Sharding: batch-parallel, one batch per NeuronCore (N=8 = n_cores), no
cross-core communication. The disk mask couples batches (all-batch AND), so
every core receives the full grid (tiny) and computes the mask locally.

Layouts (q = output pixel, 0..2303; t = q//128; p = q%128):
  xq     (2304, 512) f16 : quad-row table (row k -> 4 corner rows for lin=k).
  gcoef  (128, 36)   f32 : own-batch grid, [p, 2*t+coord].
  gall   (128, 288)  f32 : all-batch grid, [p, 16*t + 2*m + coord].
  outp   (128, 2304) f16 : [p, t*128 + c]  (host re-permutes to (c, q)).
"""

import numpy as np

N, H, W, C = 8, 48, 48, 128
HW = H * W            # 2304
NT = HW // 128        # 18
EPS = 1e-5
CLIP_HI = float(np.float32(float(H - 1) - EPS))  # 46.99999 (f32)

NCHUNK = 6            # output store granularity (finer -> smaller final store)
TPC = NT // NCHUNK    # tiles per store chunk = 3

_CACHE = {}


def _build_nc():
    from contextlib import ExitStack

    import concourse.bacc as bacc
    import concourse.bass as bass
    import concourse.mybir as mybir
    import concourse.tile as tile

    dt = mybir.dt
    f32, f16, i32 = dt.float32, dt.float16, dt.int32
    Alu = mybir.AluOpType

    nc = bacc.Bacc("TRN2", target_bir_lowering=False, debug=False, num_devices=N)

    xq = nc.dram_tensor("xq", [HW, 4 * C], f16, kind="ExternalInput")
    gcoef = nc.dram_tensor("gcoef", [128, 2 * NT], f32, kind="ExternalInput")
    gall = nc.dram_tensor("gall", [128, 16 * NT], f32, kind="ExternalInput")
    outp = nc.dram_tensor("outp", [128, HW], f16, kind="ExternalOutput")

    with tile.TileContext(nc) as tc, ExitStack() as ctx:
        pool = ctx.enter_context(tc.tile_pool(name="p", bufs=1))
        v = nc.vector
        gp = nc.gpsimd

        # ---- load grid layouts (HWDGE); gcoef first (idx critical path) ----
        g_coef = pool.tile([128, 2 * NT], f32)
        g_all = pool.tile([128, 16 * NT], f32)
        nc.sync.dma_start(g_coef[:], gcoef.ap())
        nc.sync.dma_start(g_all[:], gall.ap())

        # ---- DVE: clip + floor, fused clip/shift ---------------------------
        # cs = clip(g) - 0.5 in ONE tensor_scalar (shifted clip bounds);
        # floor = int-cast(cs): the HW cast rounds-to-nearest-even, so this
        # is exact for non-integer x; exactly-integer x may give floor-1,
        # which yields the IDENTICAL bilinear result (weight 0 ... 1 swap).
        # Fractions come out as fr = frac - 0.5; the +0.5 is folded into the
        # coefficient ops below.
        # Tile 0's index column first (separate tiles), so the first gather
        # emission starts ~2.3us earlier; the rest of the chain hides under
        # emission 0 (each SWDGE launch is ~1.4us on GPSIMD).
        cs0 = pool.tile([128, 2], f32)
        fi0 = pool.tile([128, 2], i32)
        flr0 = pool.tile([128, 2], f32)
        linf0 = pool.tile([128, 1], f32)
        idx0 = pool.tile([128, 1], i32)
        gts = []
        gt_0 = pool.tile([128, 4 * C], f16, tag="G0")
        with tc.high_priority():
            v.tensor_scalar(cs0[:], g_coef[:, 0:2], EPS, CLIP_HI, Alu.max, Alu.min)
            v.tensor_scalar(cs0[:], cs0[:], -0.5, None, Alu.add)
            v.tensor_copy(fi0[:], cs0[:])
            v.tensor_copy(flr0[:], fi0[:])
            v.scalar_tensor_tensor(
                linf0[:], flr0[:, 0:1], float(W), flr0[:, 1:2], Alu.mult, Alu.add
            )
            v.tensor_copy(idx0[:], linf0[:])
            gp.indirect_dma_start(
                out=gt_0[:],
                out_offset=None,
                in_=xq.ap(),
                in_offset=bass.IndirectOffsetOnAxis(ap=idx0[:], axis=0),
            )
        gts.append(gt_0)

        # Remaining 17 columns (fused ops), then their gather emissions.
        cs = pool.tile([128, 2 * NT], f32)
        v.tensor_scalar(cs[:, 2:], g_coef[:, 2:], EPS, CLIP_HI, Alu.max, Alu.min)
        v.tensor_scalar(cs[:, 2:], cs[:, 2:], -0.5, None, Alu.add)
        fi = pool.tile([128, 2 * NT], i32)
        v.tensor_copy(fi[:, 2:], cs[:, 2:])
        flr = pool.tile([128, 2 * NT], f32)
        v.tensor_copy(flr[:, 2:], fi[:, 2:])
        linf = pool.tile([128, NT], f32)
        v.scalar_tensor_tensor(
            linf[:, 1:], flr[:, 2::2], float(W), flr[:, 3::2], Alu.mult, Alu.add
        )
        idx = pool.tile([128, NT], i32)
        v.tensor_copy(idx[:, 1:], linf[:, 1:])

        for t in range(1, NT):
            gt_t = pool.tile([128, 4 * C], f16, tag=f"G{t}")
            gp.indirect_dma_start(
                out=gt_t[:],
                out_offset=None,
                in_=xq.ap(),
                in_offset=bass.IndirectOffsetOnAxis(ap=idx[:, t : t + 1], axis=0),
            )
            gts.append(gt_t)
        # copy tile-0's cs/flr into the full tiles for the coefficient chain
        v.tensor_copy(cs[:, 0:2], cs0[:])
        v.tensor_copy(flr[:, 0:2], flr0[:])

        # ---- DVE: disk mask = AND over batches/coords of in-bounds test ----
        g_all3 = g_all[:].rearrange("p (t m) -> p t m", m=16)
        mn = pool.tile([128, NT], f32)
        mx = pool.tile([128, NT], f32)
        v.tensor_reduce(mn[:], g_all3, mybir.AxisListType.X, Alu.min)
        v.tensor_reduce(mx[:], g_all3, mybir.AxisListType.X, Alu.max)
        mge = pool.tile([128, NT], f32)
        mle = pool.tile([128, NT], f32)
        v.tensor_scalar(mge[:], mn[:], -0.5, None, Alu.is_ge)
        v.tensor_scalar(mle[:], mx[:], float(H) - 0.5, None, Alu.is_le)
        maskf = pool.tile([128, NT], f32)
        v.tensor_tensor(maskf[:], mge[:], mle[:], Alu.mult)

        # ---- DVE: bilinear coefficients (f32, [128, NT]) -------------------
        # fr = frac - 0.5 (shifted); the +0.5 folds into the STT/TS ops.
        fr = pool.tile([128, 2 * NT], f32)
        v.tensor_tensor(fr[:], cs[:], flr[:], Alu.subtract)
        fra, frb = fr[:, 0::2], fr[:, 1::2]
        fa0 = pool.tile([128, NT], f32)      # 1-a = 0.5 - fra
        fb0 = pool.tile([128, NT], f32)      # 1-b = 0.5 - frb
        v.tensor_scalar(fa0[:], fra, -1.0, 0.5, Alu.mult, Alu.add)
        v.tensor_scalar(fb0[:], frb, -1.0, 0.5, Alu.mult, Alu.add)
        fa0m = pool.tile([128, NT], f32)     # (1-a)*mask
        fa1m = pool.tile([128, NT], f32)     # a*mask = (fra+0.5)*mask
        v.tensor_tensor(fa0m[:], fa0[:], maskf[:], Alu.mult)
        v.scalar_tensor_tensor(fa1m[:], fra, 0.5, maskf[:], Alu.add, Alu.mult)
        c00 = pool.tile([128, NT], f32)
        c01 = pool.tile([128, NT], f32)
        c10 = pool.tile([128, NT], f32)
        c11 = pool.tile([128, NT], f32)
        v.tensor_tensor(c00[:], fa0m[:], fb0[:], Alu.mult)
        v.scalar_tensor_tensor(c01[:], frb, 0.5, fa0m[:], Alu.add, Alu.mult)
        v.tensor_tensor(c10[:], fa1m[:], fb0[:], Alu.mult)
        v.scalar_tensor_tensor(c11[:], frb, 0.5, fa1m[:], Alu.add, Alu.mult)

        # ---- combine: per tile out = c00*A + c01*B + c10*C + c11*D ---------
        # ACT does two products (per-partition activation scale); DVE does
        # two fused multiply-adds + the final add. All data fp16.
        outs = []
        for k in range(NCHUNK):
            out_k = pool.tile([128, TPC * C], f16, tag=f"O{k}")
            outs.append(out_k)
            for tt in range(TPC):
                t = k * TPC + tt
                g = gts[t]
                A = g[:, 0 * C : 1 * C]
                B = g[:, 1 * C : 2 * C]
                Cr = g[:, 2 * C : 3 * C]
                D = g[:, 3 * C : 4 * C]
                u0 = pool.tile([128, C], f16, tag=f"u0_{t}")
                u1 = pool.tile([128, C], f16, tag=f"u1_{t}")
                v0 = pool.tile([128, C], f16, tag=f"v0_{t}")
                v1 = pool.tile([128, C], f16, tag=f"v1_{t}")
                nc.scalar.activation(
                    u0[:], A, mybir.ActivationFunctionType.Copy,
                    scale=c00[:, t : t + 1],
                )
                nc.scalar.activation(
                    u1[:], B, mybir.ActivationFunctionType.Copy,
                    scale=c01[:, t : t + 1],
                )
                v.scalar_tensor_tensor(
                    v0[:], Cr, c10[:, t : t + 1], u0[:], Alu.mult, Alu.add
                )
                v.scalar_tensor_tensor(
                    v1[:], D, c11[:, t : t + 1], u1[:], Alu.mult, Alu.add
                )
                v.tensor_tensor(
                    out_k[:, tt * C : (tt + 1) * C], v0[:], v1[:], Alu.add
                )
            nc.sync.dma_start(
                outp.ap()[:, k * TPC * C : (k + 1) * TPC * C], out_k[:]
            )

    nc.compile()
    return nc


def _get_nc():
    if "nc" not in _CACHE:
        _CACHE["nc"] = _build_nc()
    return _CACHE["nc"]


def _stage_inputs(x, grid):
    """Build the per-core input maps (data movement / dtype cast only)."""
    x = np.ascontiguousarray(x, dtype=np.float32)
    grid = np.ascontiguousarray(grid, dtype=np.float32)
    xr = x.reshape(N, C, HW)
    gr = grid.reshape(N, HW, 2)

    # quad-row table: xq[n][k] = [xT[k], xT[k+1], xT[k+48], xT[k+49]]  (fp16)
    xt = np.zeros((N, HW + W + 2, C), dtype=np.float16)
    xt[:, :HW] = xr.transpose(0, 2, 1)
    xq = np.empty((N, HW, 4 * C), dtype=np.float16)
    xq[:, :, 0 * C : 1 * C] = xt[:, 0 : HW]
    xq[:, :, 1 * C : 2 * C] = xt[:, 1 : HW + 1]
    xq[:, :, 2 * C : 3 * C] = xt[:, W : HW + W]
    xq[:, :, 3 * C : 4 * C] = xt[:, W + 1 : HW + W + 1]

    # gcoef[n][p, 2t+c] = gr[n, t*128+p, c]
    gc = gr.reshape(N, NT, 128, 2).transpose(0, 2, 1, 3)  # [n, p, t, c]
    gcoef = np.ascontiguousarray(gc.reshape(N, 128, 2 * NT))

    # gall[p, 16t+2m+c] = gr[m, t*128+p, c]   (same for all cores)
    ga = gr.reshape(N, NT, 128, 2).transpose(2, 1, 0, 3)  # [p, t, m, c]
    gall = np.ascontiguousarray(ga.reshape(128, 16 * NT))

    return [{"xq": xq[n], "gcoef": gcoef[n], "gall": gall} for n in range(N)]


def _unstage_output(results):
    """results[n]["outp"] is (128, 2304) f16 = [p, t*128+c] -> (N, C, H, W)."""
    out = np.empty((N, C, H, W), dtype=np.float32)
    for n in range(N):
        o = results[n]["outp"].astype(np.float32).reshape(128, NT, C)
        out[n] = o.transpose(2, 1, 0).reshape(C, H, W)   # [c, q=t*128+p]
    return out


def kernel(x, grid):
    from concourse import bass_utils

    nc = _get_nc()
    in_maps = _stage_inputs(x, grid)
    res = bass_utils.run_bass_kernel_spmd(nc, in_maps, core_ids=list(range(N)))
    return _unstage_output(res.results)


# revision 27
# speedup vs baseline: 1.0586x; 1.0586x over previous
"""BMMRemapper Trainium2 kernel.

Math: out[n,c,q] = sum_k x[n,c,k] * mat[n,q,k] where mat is the bilinear
interpolation matrix built from grid (4 nonzeros per row q: rows lin, lin+1,
lin+48, lin+49 of x^T with weights (1-a)(1-b), (1-a)b, a(1-b), ab).

Instead of a dense 2304x2304 BMM we exploit the 4-sparsity: the host stages
a quad-row table xq[k] = [x^T[k], x^T[k+1], x^T[k+48], x^T[k+49]] in fp16
(pure data movement + dtype cast), so ONE indirect-DMA descriptor per output
pixel fetches all four corner rows (1 KB contiguous). The HW vector-indirect
DMA consumes exactly one offset per partition per instruction (measured; the
simulator's multi-offset ravel semantics do NOT match HW), so the gather is
18 instructions of 128 descriptors each; their ~1.1-1.4 us SWDGE ucode
launches serialize on GPSIMD and set this kernel's floor (~25 us). The fp16
table halves the gathered bytes so the SDMA transfers, the combine (ACT: two
products via per-partition activation scale; DVE: two fused multiply-adds +
one add per tile) and the chunked output stores all hide under that wall.

Sharding: batch-parallel, one batch per NeuronCore (N=8 = n_cores), no
cross-core communication. The disk mask couples batches (all-batch AND), so
every core receives the full grid (tiny) and computes the mask locally.

Layouts (q = output pixel, 0..2303; t = q//128; p = q%128):
  xq     (2304, 512) f16 : quad-row table (row k -> 4 corner rows for lin=k).
  gcoef  (128, 36)   f32 : own-batch grid, [p, 2*t+coord].
  gall   (128, 288)  f32 : all-batch grid, [p, 16*t + 2*m + coord].
  outp   (128, 2304) f16 : [p, t*128 + c]  (host re-permutes to (c, q)).
"""

import numpy as np

N, H, W, C = 8, 48, 48, 128
HW = H * W            # 2304
NT = HW // 128        # 18
EPS = 1e-5
CLIP_HI = float(np.float32(float(H - 1) - EPS))  # 46.99999 (f32)

NCHUNK = 6            # output store granularity (finer -> smaller final store)
TPC = NT // NCHUNK    # tiles per store chunk = 3

_CACHE = {}


def _build_nc():
    from contextlib import ExitStack

    import concourse.bacc as bacc
    import concourse.bass as bass
    import concourse.mybir as mybir
    import concourse.tile as tile

    dt = mybir.dt
    f32, f16, i32 = dt.float32, dt.float16, dt.int32
    Alu = mybir.AluOpType

    nc = bacc.Bacc("TRN2", target_bir_lowering=False, debug=False, num_devices=N)

    xq = nc.dram_tensor("xq", [HW, 4 * C], f16, kind="ExternalInput")
    gcoef = nc.dram_tensor("gcoef", [128, 2 * NT], f32, kind="ExternalInput")
    gall = nc.dram_tensor("gall", [128, 16 * NT], f32, kind="ExternalInput")
    outp = nc.dram_tensor("outp", [128, HW], f16, kind="ExternalOutput")

    with tile.TileContext(nc) as tc, ExitStack() as ctx:
        pool = ctx.enter_context(tc.tile_pool(name="p", bufs=1))
        v = nc.vector
        gp = nc.gpsimd

        # ---- load grid layouts (HWDGE); gcoef first (idx critical path) ----
        g_coef = pool.tile([128, 2 * NT], f32)
        g_all = pool.tile([128, 16 * NT], f32)
        nc.sync.dma_start(g_coef[:], gcoef.ap())
        nc.sync.dma_start(g_all[:], gall.ap())

        # ---- DVE: clip + floor, fused clip/shift ---------------------------
        # cs = clip(g) - 0.5 in ONE tensor_scalar (shifted clip bounds);
        # floor = int-cast(cs): the HW cast rounds-to-nearest-even, so this
        # is exact for non-integer x; exactly-integer x may give floor-1,
        # which yields the IDENTICAL bilinear result (weight 0 ... 1 swap).
        # Fractions come out as fr = frac - 0.5; the +0.5 is folded into the
        # coefficient ops below.
        # Tile 0's index column first (separate tiles), so the first gather
        # emission starts ~2.3us earlier; the rest of the chain hides under
        # emission 0 (each SWDGE launch is ~1.4us on GPSIMD).
        cs0 = pool.tile([128, 2], f32)
        fi0 = pool.tile([128, 2], i32)
        flr0 = pool.tile([128, 2], f32)
        linf0 = pool.tile([128, 1], f32)
        idx0 = pool.tile([128, 1], i32)
        gts = []
        gt_0 = pool.tile([128, 4 * C], f16, tag="G0")
        with tc.high_priority():
            v.tensor_scalar(cs0[:], g_coef[:, 0:2], EPS, CLIP_HI, Alu.max, Alu.min)
            v.tensor_scalar(cs0[:], cs0[:], -0.5, None, Alu.add)
            v.tensor_copy(fi0[:], cs0[:])
            v.tensor_copy(flr0[:], fi0[:])
            v.scalar_tensor_tensor(
                linf0[:], flr0[:, 0:1], float(W), flr0[:, 1:2], Alu.mult, Alu.add
            )
            v.tensor_copy(idx0[:], linf0[:])
            gp.indirect_dma_start(
                out=gt_0[:],
                out_offset=None,
                in_=xq.ap(),
                in_offset=bass.IndirectOffsetOnAxis(ap=idx0[:], axis=0),
            )
        gts.append(gt_0)

        # Remaining 17 columns (fused ops), then their gather emissions.
        cs = pool.tile([128, 2 * NT], f32)
        v.tensor_scalar(cs[:, 2:], g_coef[:, 2:], EPS, CLIP_HI, Alu.max, Alu.min)
        v.tensor_scalar(cs[:, 2:], cs[:, 2:], -0.5, None, Alu.add)
        fi = pool.tile([128, 2 * NT], i32)
        v.tensor_copy(fi[:, 2:], cs[:, 2:])
        flr = pool.tile([128, 2 * NT], f32)
        v.tensor_copy(flr[:, 2:], fi[:, 2:])
        linf = pool.tile([128, NT], f32)
        v.scalar_tensor_tensor(
            linf[:, 1:], flr[:, 2::2], float(W), flr[:, 3::2], Alu.mult, Alu.add
        )
        idx = pool.tile([128, NT], i32)
        v.tensor_copy(idx[:, 1:], linf[:, 1:])

        for t in range(1, NT):
            gt_t = pool.tile([128, 4 * C], f16, tag=f"G{t}")
            gp.indirect_dma_start(
                out=gt_t[:],
                out_offset=None,
                in_=xq.ap(),
                in_offset=bass.IndirectOffsetOnAxis(ap=idx[:, t : t + 1], axis=0),
            )
            gts.append(gt_t)
        # copy tile-0's cs/flr into the full tiles for the coefficient chain
        v.tensor_copy(cs[:, 0:2], cs0[:])
        v.tensor_copy(flr[:, 0:2], flr0[:])

        # ---- DVE: disk mask = AND over batches/coords of in-bounds test ----
        g_all3 = g_all[:].rearrange("p (t m) -> p t m", m=16)
        mn = pool.tile([128, NT], f32)
        mx = pool.tile([128, NT], f32)
        v.tensor_reduce(mn[:], g_all3, mybir.AxisListType.X, Alu.min)
        v.tensor_reduce(mx[:], g_all3, mybir.AxisListType.X, Alu.max)
        mge = pool.tile([128, NT], f32)
        mle = pool.tile([128, NT], f32)
        v.tensor_scalar(mge[:], mn[:], -0.5, None, Alu.is_ge)
        v.tensor_scalar(mle[:], mx[:], float(H) - 0.5, None, Alu.is_le)
        maskf = pool.tile([128, NT], f32)
        v.tensor_tensor(maskf[:], mge[:], mle[:], Alu.mult)

        # ---- DVE: bilinear coefficients (f32, [128, NT]) -------------------
        # fr = frac - 0.5 (shifted); the +0.5 folds into the STT/TS ops.
        fr = pool.tile([128, 2 * NT], f32)
        v.tensor_tensor(fr[:], cs[:], flr[:], Alu.subtract)
        fra, frb = fr[:, 0::2], fr[:, 1::2]
        fa0 = pool.tile([128, NT], f32)      # 1-a = 0.5 - fra
        fb0 = pool.tile([128, NT], f32)      # 1-b = 0.5 - frb
        v.tensor_scalar(fa0[:], fra, -1.0, 0.5, Alu.mult, Alu.add)
        v.tensor_scalar(fb0[:], frb, -1.0, 0.5, Alu.mult, Alu.add)
        fa0m = pool.tile([128, NT], f32)     # (1-a)*mask
        fa1m = pool.tile([128, NT], f32)     # a*mask = (fra+0.5)*mask
        v.tensor_tensor(fa0m[:], fa0[:], maskf[:], Alu.mult)
        v.scalar_tensor_tensor(fa1m[:], fra, 0.5, maskf[:], Alu.add, Alu.mult)
        c00 = pool.tile([128, NT], f32)
        c01 = pool.tile([128, NT], f32)
        c10 = pool.tile([128, NT], f32)
        c11 = pool.tile([128, NT], f32)
        v.tensor_tensor(c00[:], fa0m[:], fb0[:], Alu.mult)
        v.scalar_tensor_tensor(c01[:], frb, 0.5, fa0m[:], Alu.add, Alu.mult)
        v.tensor_tensor(c10[:], fa1m[:], fb0[:], Alu.mult)
        v.scalar_tensor_tensor(c11[:], frb, 0.5, fa1m[:], Alu.add, Alu.mult)

        # ---- combine: per tile out = c00*A + c01*B + c10*C + c11*D ---------
        # ACT does two products (per-partition activation scale); DVE does
        # two fused multiply-adds + the final add. All data fp16.
        outs = []
        for k in range(NCHUNK):
            out_k = pool.tile([128, TPC * C], f16, tag=f"O{k}")
            outs.append(out_k)
            for tt in range(TPC):
                t = k * TPC + tt
                g = gts[t]
                A = g[:, 0 * C : 1 * C]
                B = g[:, 1 * C : 2 * C]
                Cr = g[:, 2 * C : 3 * C]
                D = g[:, 3 * C : 4 * C]
                u0 = pool.tile([128, C], f16, tag=f"u0_{t}")
                u1 = pool.tile([128, C], f16, tag=f"u1_{t}")
                v0 = pool.tile([128, C], f16, tag=f"v0_{t}")
                v1 = pool.tile([128, C], f16, tag=f"v1_{t}")
                if t < 4:
                    # first tiles' products on DVE (idle at early data
                    # arrival): trims ACT's queue so a late ACT start can
                    # never backlog into the final tiles (tail variance)
                    v.tensor_scalar(u0[:], A, c00[:, t : t + 1], None, Alu.mult)
                    v.tensor_scalar(u1[:], B, c01[:, t : t + 1], None, Alu.mult)
                else:
                    nc.scalar.activation(
                        u0[:], A, mybir.ActivationFunctionType.Copy,
                        scale=c00[:, t : t + 1],
                    )
                    nc.scalar.activation(
                        u1[:], B, mybir.ActivationFunctionType.Copy,
                        scale=c01[:, t : t + 1],
                    )
                v.scalar_tensor_tensor(
                    v0[:], Cr, c10[:, t : t + 1], u0[:], Alu.mult, Alu.add
                )
                v.scalar_tensor_tensor(
                    v1[:], D, c11[:, t : t + 1], u1[:], Alu.mult, Alu.add
                )
                v.tensor_tensor(
                    out_k[:, tt * C : (tt + 1) * C], v0[:], v1[:], Alu.add
                )
            nc.sync.dma_start(
                outp.ap()[:, k * TPC * C : (k + 1) * TPC * C], out_k[:]
            )

    nc.compile()
    return nc


def _get_nc():
    if "nc" not in _CACHE:
        _CACHE["nc"] = _build_nc()
    return _CACHE["nc"]


def _stage_inputs(x, grid):
    """Build the per-core input maps (data movement / dtype cast only)."""
    x = np.ascontiguousarray(x, dtype=np.float32)
    grid = np.ascontiguousarray(grid, dtype=np.float32)
    xr = x.reshape(N, C, HW)
    gr = grid.reshape(N, HW, 2)

    # quad-row table: xq[n][k] = [xT[k], xT[k+1], xT[k+48], xT[k+49]]  (fp16)
    xt = np.zeros((N, HW + W + 2, C), dtype=np.float16)
    xt[:, :HW] = xr.transpose(0, 2, 1)
    xq = np.empty((N, HW, 4 * C), dtype=np.float16)
    xq[:, :, 0 * C : 1 * C] = xt[:, 0 : HW]
    xq[:, :, 1 * C : 2 * C] = xt[:, 1 : HW + 1]
    xq[:, :, 2 * C : 3 * C] = xt[:, W : HW + W]
    xq[:, :, 3 * C : 4 * C] = xt[:, W + 1 : HW + W + 1]

    # gcoef[n][p, 2t+c] = gr[n, t*128+p, c]
    gc = gr.reshape(N, NT, 128, 2).transpose(0, 2, 1, 3)  # [n, p, t, c]
    gcoef = np.ascontiguousarray(gc.reshape(N, 128, 2 * NT))

    # gall[p, 16t+2m+c] = gr[m, t*128+p, c]   (same for all cores)
    ga = gr.reshape(N, NT, 128, 2).transpose(2, 1, 0, 3)  # [p, t, m, c]
    gall = np.ascontiguousarray(ga.reshape(128, 16 * NT))

    return [{"xq": xq[n], "gcoef": gcoef[n], "gall": gall} for n in range(N)]


def _unstage_output(results):
    """results[n]["outp"] is (128, 2304) f16 = [p, t*128+c] -> (N, C, H, W)."""
    out = np.empty((N, C, H, W), dtype=np.float32)
    for n in range(N):
        o = results[n]["outp"].astype(np.float32).reshape(128, NT, C)
        out[n] = o.transpose(2, 1, 0).reshape(C, H, W)   # [c, q=t*128+p]
    return out


def kernel(x, grid):
    from concourse import bass_utils

    nc = _get_nc()
    in_maps = _stage_inputs(x, grid)
    res = bass_utils.run_bass_kernel_spmd(nc, in_maps, core_ids=list(range(N)))
    return _unstage_output(res.results)
